# revision 31
# baseline (speedup 1.0000x reference)
"""Multi-head attention (B=2, S=2048, E=768, H=12, D=64) on 8 NeuronCores.

Sharding: core c -> batch b = c//4, head group hg = c%4 (3 heads each).
Each core computes the qkv projection for its 3 heads, attention, and a
partial output projection (rows of w_proj for its heads). Host sums the 4
partials per batch and adds the bias terms (tensor-parallel unshard).

Device dataflow (everything transposed so no on-chip transposes are
needed, and every matmul keeps the full 128-partition contraction busy —
K=64 matmuls halve PE-array activity and trip the HAM clock governor):
  xT [768, 2048] fp16 (host-pretransposed), w [768, 576] = [A|B|C|V],
  A = [q0|q1], B = [k0|k1], C = [k2|q2], V = [v0|v1|v2].
  qkT tiles t0=[q0|q1], t1=[k0|0], t2=[0|k1], t3=[0|q2], t4=[0|k2]:
  each head's scoresT matmul uses a full-128-partition stationary whose
  zero half kills the other head's rows.
  pT = exp(scoresT/8) on ScalarE — the ~110us exp stream paces the
  kernel; ScalarE does nothing else until the output phase.
  v' = [v(64) | ones(64)] per Sk block: the AV matmul also produces the
  softmax denominator in rows 64:128.
  AV h0/h1: two PSUM chains per Sq-chunk pair, fed Sk-major in half-sweeps
  woven into the next head's scores loop — pT tiles free progressively,
  so the pT pool never stalls the exp stream.
  AV h2: 4-block PSUM partials added into an SBUF fp16 accumulator (DVE),
  woven one window behind their exp gates, so the chains ride the h2 exp
  stream instead of serializing after it.
  outT = avT[0:64] * reciprocal_approx_fast(denominator)  (DVE)
  yT = [wpA; wpB(pad)]^T @ outT per (mt, nq) chunk, staged fp16, DMA out.

Schedule: PE warms up on a memset tile while the DMAs spin up (the HAM
clock ramps during the ~7us framework preamble + first transfers); inputs
arrive as 8 large multi-level-AP DMAs ordered by first use (bias, w[A|B],
xT in column quarters/half, w[C|V], wp); head-0 scores run all 16 Sk
blocks on q-columns 0:1024 first so the exp stream starts as early as the
DMA pipe allows; projection and v' fillers are woven between scores steps
at a rate matched to the ScalarE exp pace, with dummy warm matmuls
filling the DMA-starved first steps to keep the PE clock up.
"""

import threading

import numpy as np

import concourse.bass as bass
import concourse.tile as tile
from concourse import bacc, mybir
from concourse.bass import ts, ds
from concourse.bass_utils import run_bass_kernel_spmd

F32 = mybir.dt.float32
F16 = mybir.dt.float16

EMBED = 768
NH = 12
D = 64
B = 2
S = 2048
HPC = 3          # heads per core
NCORES = 8
P = 128
KC = EMBED // P  # 6 contraction chunks
NQ = S // 512    # 4 Sq chunks of 512
NSK = S // P     # 16 Sk blocks
WC = 576         # w columns per k-chunk: A(128) B(128) C(128) V(192)


def _build_kernel(nc):
    w_in = nc.dram_tensor("w_in", [EMBED, WC], F16, kind="ExternalInput").ap()
    b_in = nc.dram_tensor("b_in", [P, 4], F32, kind="ExternalInput").ap()
    wpA_in = nc.dram_tensor("wpA", [P, EMBED], F16, kind="ExternalInput").ap()
    wpB_in = nc.dram_tensor("wpB", [P, EMBED], F16, kind="ExternalInput").ap()
    xT_in = nc.dram_tensor("xT", [EMBED, S], F16, kind="ExternalInput").ap()
    yT = nc.dram_tensor("yT", [EMBED, S], F16, kind="ExternalOutput").ap()

    with tile.TileContext(nc) as tc:
        with (
            tc.tile_pool(name="wpool", bufs=1) as wpool,
            tc.tile_pool(name="qkpool", bufs=1) as qkpool,
            tc.tile_pool(name="ptpool", bufs=27) as ptpool,
            tc.tile_pool(name="opool", bufs=1) as opool,
            tc.tile_pool(name="rlpool", bufs=2) as rlpool,
            tc.tile_pool(name="psum", bufs=3, space="PSUM") as psum,
        ):
            # ---- t0 memsets: warm tile on DVE (PE warmup gate), the rest
            # on GPSIMD so they never serialize ahead of the DVE bias-adds
            warm = wpool.tile([P, 512], F16, name="warm")
            nc.vector.memset(warm, 0.0)
            qkT = [qkpool.tile([P, S], F16, name=f"qkT{t}") for t in range(5)]
            for t in (1, 2, 3, 4):
                zero = slice(D, P) if t == 1 else slice(0, D)
                nc.gpsimd.memset(qkT[t][zero, :], 0.0)
            vp = []
            for h in range(HPC):
                vp_h = wpool.tile([P, NSK * P], F16, name=f"vp{h}")
                nc.gpsimd.memset(
                    vp_h.rearrange("p (s c) -> p s c", c=P)[:, :, D:P], 1.0
                )
                vp.append(vp_h)
            st01 = opool.tile([P, S], F16, name="st01")
            outT2 = opool.tile([P, S], F16, name="outT2")
            nc.gpsimd.memset(outT2[D:P, :], 0.0)
            avacc = opool.tile([P, S], F16, name="avacc")

            # ACT: preload the Exp table before the stream needs it
            wexp = rlpool.tile([P, 1], F32, name="wexp", tag="wexp", bufs=1)
            nc.scalar.activation(
                out=wexp, in_=warm[:, 0:1],
                func=mybir.ActivationFunctionType.Exp, scale=1.0,
            )

            # ---- input DMAs: large multi-level-AP transfers, issued in
            # first-use order (the DMA pipe sustains only ~200GB/s, so the
            # order IS the head-phase critical path) ----
            w_sb = wpool.tile([P, KC * WC], F16, name="w_sb")
            wv_out = w_sb.rearrange("p (k c) -> p k c", c=WC)
            wv_in = w_in.rearrange("(k p) c -> p k c", p=P)
            nc.sync.dma_start(out=wv_out[:, :, 0:256], in_=wv_in[:, :, 0:256])
            xT_sb = wpool.tile([P, KC * S], F16, name="xT_sb")
            xv_out = xT_sb.rearrange("p (k s) -> p k s", s=S)
            xv_in = xT_in.rearrange("(k p) s -> p k s", p=P)
            nc.sync.dma_start(out=xv_out[:, :, 0:512], in_=xv_in[:, :, 0:512])
            nc.sync.dma_start(out=xv_out[:, :, 512:1024], in_=xv_in[:, :, 512:1024])
            bias = wpool.tile([P, 4], F32, name="bias")
            nc.sync.dma_start(out=bias, in_=b_in)
            nc.sync.dma_start(out=xv_out[:, :, 1024:S], in_=xv_in[:, :, 1024:S])
            nc.sync.dma_start(out=wv_out[:, :, 256:WC], in_=wv_in[:, :, 256:WC])
            wpA = wpool.tile([P, EMBED], F16, name="wpA")
            nc.sync.dma_start(out=wpA, in_=wpA_in)
            wpB = wpool.tile([P, EMBED], F16, name="wpB")
            nc.sync.dma_start(out=wpB, in_=wpB_in)

            # ---- PE warmup on the memset tile (ramps the HAM clock while
            # the DMAs stream; no data dependency) ----
            wps = psum.tile([P, 512], F32, tag="av", bufs=2, name="warmps")
            for r in range(8):
                nc.tensor.matmul(
                    wps, lhsT=warm[:, 0:128], rhs=warm,
                    start=(r == 0), stop=(r == 7),
                )
            wps2 = psum.tile([P, 512], F32, tag="av", bufs=2, name="warmps2")
            for r in range(4):
                nc.tensor.matmul(
                    wps2, lhsT=warm[:, 0:128], rhs=warm,
                    start=(r == 0), stop=(r == 3),
                )

            def warm_fill(n):
                """Dummy full-array matmuls: keep the PE busy (and the HAM
                clock up) through DMA-starved windows."""
                ps = psum.tile([P, 512], F32, tag="av", bufs=2, name="warmf")
                for r in range(n):
                    nc.tensor.matmul(
                        ps, lhsT=warm[:, 0:128], rhs=warm,
                        start=(r == 0), stop=(r == n - 1),
                    )

            # ---- qkv projection (PSUMs ride the "av" tag, idle during
            # the scores loops, so they never stall the "sc" rotation
            # that paces the exp stream) ----
            def proj_chunk(t, nq):
                ps = psum.tile([P, 512], F32, tag="av", bufs=2, name="ps_qk")
                for k in range(KC):
                    nc.tensor.matmul(
                        ps,
                        lhsT=w_sb[:, ds(k * WC + t * P, P)],
                        rhs=xT_sb[:, ds(k * S + nq * 512, 512)],
                        start=(k == 0),
                        stop=(k == KC - 1),
                    )
                c = ts(nq, 512)
                if t == 0:
                    nc.vector.tensor_scalar_add(
                        out=qkT[0][:, c], in0=ps, scalar1=bias[:, 0:1]
                    )
                elif t == 1:
                    # B = [k0|k1] -> t1 rows 0:64, t2 rows 64:128
                    nc.vector.tensor_scalar_add(
                        out=qkT[1][0:D, c], in0=ps[0:D, :], scalar1=bias[0:D, 1:2]
                    )
                    nc.vector.tensor_scalar_add(
                        out=qkT[2][D:P, c], in0=ps[D:P, :], scalar1=bias[D:P, 1:2]
                    )
                else:
                    # C = [k2|q2] -> t4 rows 64:128 (shift), t3 rows 64:128
                    nc.vector.tensor_scalar_add(
                        out=qkT[4][D:P, c], in0=ps[0:D, :], scalar1=bias[0:D, 2:3]
                    )
                    nc.vector.tensor_scalar_add(
                        out=qkT[3][D:P, c], in0=ps[D:P, :], scalar1=bias[D:P, 3:4]
                    )

            def v_chunk(st):
                vps = psum.tile([P, HPC * D], F32, tag="av", bufs=2, name="ps_v")
                for k in range(KC):
                    nc.tensor.matmul(
                        vps,
                        lhsT=xT_sb[:, ds(k * S + st * P, P)],
                        rhs=w_sb[:, ds(k * WC + 384, HPC * D)],
                        start=(k == 0),
                        stop=(k == KC - 1),
                    )
                for h in range(HPC):
                    nc.vector.tensor_copy(
                        out=vp[h][:, ds(st * P, D)], in_=vps[:, ts(h, D)]
                    )

            # ---- scores + exp (the ScalarE pacer) ----
            pts = [[], [], []]
            KT = {0: 1, 1: 2, 2: 4}
            QT = {0: 0, 1: 0, 2: 3}

            def scores_g(h, sk, g):
                if g == 0:
                    pt = ptpool.tile([P, S], F16, name="pt", tag="pt")
                    pts[h].append(pt)
                pt = pts[h][sk]
                kt, qt = qkT[KT[h]], qkT[QT[h]]
                sps = psum.tile([P, 1024], F32, tag="sc", name="ps_s")
                for j in range(2):
                    nc.tensor.matmul(
                        sps[:, ts(j, 512)],
                        lhsT=kt[:, ts(sk, P)],
                        rhs=qt[:, ds(g * 1024 + j * 512, 512)],
                        start=True,
                        stop=True,
                    )
                nc.scalar.activation(
                    out=pt[:, ts(g, 1024)],
                    in_=sps,
                    func=mybir.ActivationFunctionType.Exp,
                    scale=float(D) ** -0.5,
                )

            # ---- AV + normalize ----
            def norm_psum(h, nq, av):
                dst = st01[0:D, ts(nq, 512)] if h == 0 else st01[D:P, ts(nq, 512)]
                ll = rlpool.tile([D, 512], F32, name="ll", tag="ll")
                nc.vector.tensor_copy(out=ll, in_=av[D:P, :])
                rr = rlpool.tile([D, 512], F32, name="rr", tag="rr")
                nc.vector.reciprocal_approx_fast(out=rr, in_=ll)
                nc.vector.tensor_mul(out=dst, in0=av[0:D, :], in1=rr)

            def av_sweep(h, nqa, nqb, lo, hi, avs):
                """Feed the two open AV chains (Sq chunks nqa/nqb) Sk-major
                for blocks [lo, hi) — pT tiles free progressively."""
                for sk in range(lo, hi):
                    for q, av in ((nqa, avs[0]), (nqb, avs[1])):
                        nc.tensor.matmul(
                            av,
                            lhsT=vp[h][:, ts(sk, P)],
                            rhs=pts[h][sk][:, ts(q, 512)],
                            start=(sk == 0),
                            stop=(sk == NSK - 1),
                        )

            def av_pair_open():
                return [
                    psum.tile([P, 512], F32, tag="av", bufs=2, name="ps_av")
                    for _ in range(2)
                ]

            def av2_partial(nq, j):
                """h2: 4-block PSUM partial accumulated into SBUF fp16."""
                ps = psum.tile([P, 512], F32, tag="sc", name="ps_av2")
                for i in range(4):
                    sk = 4 * j + i
                    nc.tensor.matmul(
                        ps,
                        lhsT=vp[2][:, ts(sk, P)],
                        rhs=pts[2][sk][:, ts(nq, 512)],
                        start=(i == 0),
                        stop=(i == 3),
                    )
                dst = avacc[:, ts(nq, 512)]
                if j == 0:
                    nc.vector.tensor_copy(out=dst, in_=ps)
                else:
                    nc.vector.tensor_add(out=dst, in0=ps, in1=dst)

            def norm2(nq):
                # tail normalize: spread across ACT (idle after the exp
                # stream) / DVE / GPSIMD so the DVE never serializes it
                ll = rlpool.tile([D, 512], F32, name="ll", tag="ll")
                nc.scalar.copy(out=ll, in_=avacc[D:P, ts(nq, 512)])
                rr = rlpool.tile([D, 512], F32, name="rr", tag="rr")
                nc.vector.reciprocal_approx_fast(out=rr, in_=ll)
                nc.gpsimd.tensor_mul(
                    out=outT2[0:D, ts(nq, 512)],
                    in0=avacc[0:D, ts(nq, 512)],
                    in1=rr,
                )

            # ---- output projection: mt-pairs staged into one [128,1024]
            # tile (ACT + DVE copies in parallel), one Sync DMA per pair
            yv = yT.rearrange("(m p) s -> p m s", p=P)

            def y_chunk(nq):
                for mp in range(3):
                    ysb = rlpool.tile([P, 1024], F16, name="ysb", tag="ysb", bufs=4)
                    for half in range(2):
                        mt = 2 * mp + half
                        yps = psum.tile(
                            [P, 512], F32, name="ps_y",
                            tag="sc" if half == 0 else "av",
                            bufs=3 if half == 0 else 2,
                        )
                        nc.tensor.matmul(
                            yps,
                            lhsT=wpA[:, ts(mt, P)],
                            rhs=st01[:, ts(nq, 512)],
                            start=True,
                            stop=False,
                        )
                        nc.tensor.matmul(
                            yps,
                            lhsT=wpB[:, ts(mt, P)],
                            rhs=outT2[:, ts(nq, 512)],
                            start=False,
                            stop=True,
                        )
                        dst = ysb[:, ts(half, 512)]
                        if half == 0:
                            nc.scalar.copy(out=dst, in_=yps)
                        else:
                            nc.vector.tensor_copy(out=dst, in_=yps)
                    nc.sync.dma_start(
                        out=yv[:, 2 * mp: 2 * mp + 2, ts(nq, 512)],
                        in_=ysb.rearrange("p (m s) -> p m s", s=512),
                    )

            def scores_q(h, sk, q):
                """512-wide scores+exp for the first Sk blocks: lets the
                exp stream start on xq0 alone, before xq1 lands."""
                if q == 0:
                    pt = ptpool.tile([P, S], F16, name="pt", tag="pt")
                    pts[h].append(pt)
                pt = pts[h][sk]
                kt, qt = qkT[KT[h]], qkT[QT[h]]
                sps = psum.tile([P, 1024], F32, tag="sc", name="ps_s")
                nc.tensor.matmul(
                    sps[:, 0:512],
                    lhsT=kt[:, ts(sk, P)],
                    rhs=qt[:, ts(q, 512)],
                    start=True,
                    stop=True,
                )
                nc.scalar.activation(
                    out=pt[:, ts(q, 512)],
                    in_=sps[:, 0:512],
                    func=mybir.ActivationFunctionType.Exp,
                    scale=float(D) ** -0.5,
                )

            # ================= emission schedule =================
            # prime head-0 scores: t0/t1 over q columns 0:512 only
            proj_chunk(0, 0)
            proj_chunk(1, 0)

            # first Sk blocks at 512-col granularity (xq0-gated only)
            for sk in range(4):
                scores_q(0, sk, 0)
                if sk == 0:
                    proj_chunk(0, 1)
                elif sk == 1:
                    proj_chunk(1, 1)
            for sk in range(4):
                scores_q(0, sk, 1)

            # rest of the h0 g0 sweep at 1024-col granularity
            fillers_a = {5: (1, 2), 7: (1, 3), 9: (0, 2),
                         11: (0, 3), 13: (2, 0), 15: (2, 1)}
            for sk in range(4, NSK):
                if sk in fillers_a:
                    proj_chunk(*fillers_a[sk])
                scores_g(0, sk, 0)
            # h0 g1 sweep, remaining proj + v' fillers woven
            fillers_b = [(2, 2), (2, 3)]
            for sk in range(NSK):
                if sk < 2:
                    proj_chunk(*fillers_b[sk])
                else:
                    v_chunk(sk - 2)
                scores_g(0, sk, 1)
            v_chunk(14)
            v_chunk(15)

            # AV rides the NEXT head's scores loop as Sk-major quarter-
            # sweeps, one per window — smooth PE duty, progressive pT
            # frees, and (for the h2 partials) gates that are a window old.
            def av_weave(h, sk, avs):
                qa, qb = (0, 1) if sk < 8 else (2, 3)
                key = (h, qa)
                if sk % 8 == 1:
                    avs[key] = av_pair_open()
                av_sweep(h, qa, qb, (sk % 8 // 2) * 4, (sk % 8 // 2 + 1) * 4,
                         avs[key])
                if sk % 8 == 7:
                    norm_psum(h, qa, avs[key][0])
                    norm_psum(h, qb, avs[key][1])

            # h1 scores with AV h0 woven; the sk==15 weave is deferred past
            # the first h2 scores so the phase boundary never gaps the
            # exp stream
            avs = {}
            for sk in range(NSK):
                scores_g(1, sk, 0)
                scores_g(1, sk, 1)
                if sk % 2 == 1 and sk < 15:
                    av_weave(0, sk, avs)

            # h2 scores with AV h1 quarter-sweeps + AV h2 partials, two
            # per window and a window behind their exp gates, so neither
            # the PE stream nor the "sc" PSUM rotation ever blocks on them
            av2_sched = {4: (0, 0), 6: (0, 2), 8: (1, 0), 10: (1, 2),
                         12: (2, 0), 14: (2, 2)}
            for sk in range(NSK):
                scores_g(2, sk, 0)
                scores_g(2, sk, 1)
                if sk == 0:
                    av_weave(0, 15, avs)
                if sk % 2 == 1 and sk < 15:
                    av_weave(1, sk, avs)
                if sk in av2_sched:
                    j, nq0 = av2_sched[sk]
                    av2_partial(nq0, j)
                    av2_partial(nq0 + 1, j)
            av_weave(1, 15, avs)

            # tail: last AV h2 partial group, normalize h2, project out.
            # Warm fills bridge the adds/norms latency pocket so the HAM
            # clock stays at full speed for the y matmuls.
            for nq in range(NQ):
                av2_partial(nq, 3)
                norm2(nq)
            warm_fill(6)
            for nq in range(NQ):
                y_chunk(nq)
    return nc


_CACHE = threading.Lock(), {}


def _get_nc():
    lock, cache = _CACHE
    with lock:
        if "nc" not in cache:
            nc = bacc.Bacc("TRN2", target_bir_lowering=False, debug=False)
            _build_kernel(nc)
            nc.compile()
            cache["nc"] = nc
        return cache["nc"]


def _shard_inputs(x, w_qkv, b_qkv, w_proj):
    """Build the 8 per-core input maps (host-side sharding/layout)."""
    E = EMBED
    in_maps = []
    for c in range(NCORES):
        b = c // 4
        hg = c % 4
        h0 = HPC * hg
        qc = [np.s_[D * (h0 + i): D * (h0 + i + 1)] for i in range(HPC)]
        kc = [np.s_[E + D * (h0 + i): E + D * (h0 + i + 1)] for i in range(HPC)]
        vc = [np.s_[2 * E + D * (h0 + i): 2 * E + D * (h0 + i + 1)] for i in range(HPC)]

        # w = [A|B|C|V]: A=[q0|q1], B=[k0|k1], C=[k2|q2], V=[v0|v1|v2]
        w = np.concatenate(
            [
                w_qkv[:, qc[0]], w_qkv[:, qc[1]],
                w_qkv[:, kc[0]], w_qkv[:, kc[1]],
                w_qkv[:, kc[2]], w_qkv[:, qc[2]],
                w_qkv[:, vc[0]], w_qkv[:, vc[1]], w_qkv[:, vc[2]],
            ],
            axis=1,
        )
        # bias [128, 4]: col0=[bq0;bq1], col1=[bk0;bk1],
        # col2=[bk2;-] (rows 0:64), col3=[-;bq2] (rows 64:128)
        bia = np.zeros((P, 4), dtype=np.float32)
        bia[0:D, 0] = b_qkv[qc[0]]
        bia[D:P, 0] = b_qkv[qc[1]]
        bia[0:D, 1] = b_qkv[kc[0]]
        bia[D:P, 1] = b_qkv[kc[1]]
        bia[0:D, 2] = b_qkv[kc[2]]
        bia[D:P, 3] = b_qkv[qc[2]]

        wpa = w_proj[D * h0: D * h0 + P]          # heads 0,1 rows
        wpb = np.zeros((P, E), dtype=np.float32)  # head 2 rows, zero-padded
        wpb[0:D] = w_proj[D * h0 + P: D * (h0 + HPC)]
        in_maps.append(
            {
                "w_in": w.astype(np.float16),
                "b_in": bia,
                "wpA": np.ascontiguousarray(wpa).astype(np.float16),
                "wpB": wpb.astype(np.float16),
                "xT": np.ascontiguousarray(x[b].T).astype(np.float16),
            }
        )
    return in_maps


def kernel(x, w_qkv, b_qkv, w_proj, b_proj, _results_hook=None):
    x = np.asarray(x, dtype=np.float32)
    w_qkv = np.asarray(w_qkv, dtype=np.float32)
    b_qkv = np.asarray(b_qkv, dtype=np.float32)
    w_proj = np.asarray(w_proj, dtype=np.float32)
    b_proj = np.asarray(b_proj, dtype=np.float32)

    nc = _get_nc()
    in_maps = _shard_inputs(x, w_qkv, b_qkv, w_proj)
    res = run_bass_kernel_spmd(nc, in_maps, core_ids=list(range(NCORES)))
    if _results_hook is not None:
        _results_hook(res)

    # unshard: sum the 4 head-group partials per batch, add bias terms
    # (v-bias contributes exactly b_v @ w_proj because attn rows sum to 1)
    b_v = b_qkv[2 * EMBED:]
    bias_row = b_v @ w_proj + b_proj  # [768]
    out = np.empty((B, S, EMBED), dtype=np.float32)
    for b in range(B):
        acc = np.zeros((EMBED, S), dtype=np.float32)
        for hg in range(4):
            acc += res.results[4 * b + hg]["yT"].astype(np.float32)
        out[b] = acc.T + bias_row
    return out
